# revision 34
# baseline (speedup 1.0000x reference)
"""Trainium2 Bass kernel for MultiHeadLatentAttention (MLA), 8-core SPMD.

Sharding: core c = (batch b=c//4, head-group g=c%4 of 4 heads).
Each core computes the full latent down-projections for its batch
(replicated across the 4 cores of that batch), head-sharded
up-projections + RoPE + causal attention, and a partial o-projection
(its heads' rows of Wo). The host sums the 4 partial outputs per batch.

Shapes (fixed): B=2, S=2048, H=2048, L=256, nh=16, hd=128, rd=64.

v9 design:
- hs arrives PRE-TRANSPOSED from the host as hsT [H, S] in bf16 (plus
  the H-contraction weights), so phase A is a pure bf16 matmul stream.
  The latents kv_d/q_d are kept in f32r; everything the attention
  matmuls touch (kT/qT/v/es/yT/wo) is bf16 (same PE speed, half the
  SBUF/DMA, 2x DVE).  PSUM stays f32 throughout.
- ONE psum pool (tags: 2-bank "blk" x2, py x2, pden x2 = 8 banks) spans
  B/C1/C2/D with no pool barriers.  Transient projection psums ride the
  blk ring (fast evacuations only); the q-rope chain keeps its own psum
  via the blk ring too but is emitted early so its DVE latency hides.
- C2 is ONE continuous unit stream (unit = head x 2 k-blocks): scores
  into a [128,1024] blk tile, one exp per unit (ACT carries nothing
  else), causal mask via affine_select, AV + ones-matmul denominator
  issued 3 units behind, reciprocal_approx_fast + tensor_mul normalize
  straight from PSUM.  Pair 1's B/C1 prep is emitted in small pieces
  INTERLEAVED among pair 0's attention units, so its matmuls fill the
  same PE stream and its rope DVE chains drain under attention compute.
- D accumulates o-proj in two blk tiles per 128-row chunk; bf16 output
  is summed across cores on the host.
"""

import sys
import os

sys.path.insert(0, "/opt/trn_rl_repo")

import numpy as np

B = 2
S = 2048
H = 2048
L = 256          # latent dim (2 chunks of 128)
NH = 16          # total heads
HPC = 4          # heads per core
HD = 128         # head dim
RD = 64          # rope / content half-width
ROPE_BASE = 10000.0
SCALE = float(HD) ** -0.5

SB = 512         # s-block for phase A / q-blocks in attention
KB = 128         # k-block in attention
NKC = H // 128   # 16 contraction chunks over H
NLC = L // 128   # 2 contraction chunks over L

MM_DTYPE = os.environ.get("MLA_MM_DTYPE", "f32r")
A_BF16 = os.environ.get("MLA_A_BF16", "1") == "1"
ATTN_BF16 = os.environ.get("MLA_ATTN_BF16", "1") == "1"
DEBUG = os.environ.get("MLA_DEBUG", "0") == "1"


def build_nc(s=S, mm_dtype=MM_DTYPE, a_bf16=A_BF16, attn_bf16=ATTN_BF16):
    """Build the Bass module for one core."""
    from concourse import bacc
    import concourse.bass as bass
    import concourse.mybir as mybir
    import concourse.tile as tile
    from contextlib import ExitStack

    f32 = mybir.dt.float32
    f32r = mybir.dt.float32r
    bf16 = mybir.dt.bfloat16
    mdt = f32r if mm_dtype == "f32r" else f32
    adt = bf16 if a_bf16 else mdt       # phase-A matmul dtype
    xdt = bf16 if attn_bf16 else mdt    # attention-side matmul dtype

    nsb = s // SB        # 512-wide s-blocks (phase A)
    nsc = s // 128       # 128-row s-chunks
    nsb2 = s // 1024     # 1024-wide blocks in C1
    assert s % 2048 == 0

    nc = bacc.Bacc(None, target_bir_lowering=False)

    hst = nc.dram_tensor("hst", [H, s], adt, kind="ExternalInput")
    w_down = nc.dram_tensor("w_down", [H, 512], adt, kind="ExternalInput")
    w_rk = nc.dram_tensor("w_rk", [H, HPC * RD], adt, kind="ExternalInput")
    w_qc = nc.dram_tensor("w_qc", [L, HPC * RD], mdt, kind="ExternalInput")
    w_qr = nc.dram_tensor("w_qr", [L, HPC * RD], mdt, kind="ExternalInput")
    w_ku = nc.dram_tensor("w_ku", [L, HPC * RD], mdt, kind="ExternalInput")
    w_vu = nc.dram_tensor("w_vu", [L, HPC * HD], mdt, kind="ExternalInput")
    w_o = nc.dram_tensor("w_o", [HPC * HD, H], xdt, kind="ExternalInput")
    # cos/sin halves replicated across all four 32-partition quadrants so any
    # 32-row operand pairs with a table slice at the SAME base partition.
    rope_cc = nc.dram_tensor("rope_cc", [128, s], f32, kind="ExternalInput")
    rope_ss = nc.dram_tensor("rope_ss", [128, s], f32, kind="ExternalInput")
    out = nc.dram_tensor("out", [s, H], bf16, kind="ExternalOutput")

    Exp = mybir.ActivationFunctionType.Exp
    is_ge = mybir.AluOpType.is_ge

    dbg = {}
    if DEBUG:
        for nm, shape in (("d_kvd", [128, NLC, s]), ("d_qd", [128, NLC, s]),
                          ("d_krt", [128, 2, s]), ("d_qT", [128, HPC, s]),
                          ("d_kT", [128, HPC, s]),
                          ("d_v", [128, 2, (s // 128) * 2 * HD]),
                          ("d_yT", [128, HPC, s])):
            dbg[nm] = nc.dram_tensor(nm, shape, f32, kind="ExternalOutput")

    hst_r = hst.rearrange("(ko p) m -> p ko m", p=128)

    with ExitStack() as top:
        tc = top.enter_context(tile.TileContext(nc))

        const_pool = top.enter_context(
            tc.tile_pool(name="const", bufs=1, side="right"))
        ones128 = const_pool.tile([128, 128], xdt, tag="ones")
        ones_f32 = const_pool.tile([128, 128], f32, tag="ones_f32")
        nc.gpsimd.memset(ones_f32[:], 1.0)
        nc.vector.tensor_copy(ones128[:], ones_f32[:])
        # preload the exp table-set now so C2's first exp doesn't pay the
        # ~2.7us ACT_TABLE_LOAD (scalar queue carries no DMAs, so this
        # does not delay anything)
        actwarm = const_pool.tile([128, 128], f32, tag="actwarm")
        nc.scalar.activation(actwarm[:], ones_f32[:], Exp)

        # pools that outlive phase A but close mid-C2 (manual mgmt, left)
        rope_cm = tc.tile_pool(name="ropes", bufs=1)
        rope_pool = rope_cm.__enter__()
        cc_t = rope_pool.tile([128, s], f32, tag="ropec")
        ss_t = rope_pool.tile([128, s], f32, tag="ropes")

        wsmall_cm = tc.tile_pool(name="wsmall", bufs=1)
        wsmall = wsmall_cm.__enter__()
        w_qc_t = wsmall.tile([128, NLC, HPC * RD], mdt, tag="wqc")
        w_qr_t = wsmall.tile([128, NLC, HPC * RD], mdt, tag="wqr")
        w_ku_t = wsmall.tile([128, NLC, HPC * RD], mdt, tag="wku")
        w_vu_t = wsmall.tile([128, NLC, HPC * HD], mdt, tag="wvu")

        lat_cm = tc.tile_pool(name="lat", bufs=1)
        lat = lat_cm.__enter__()
        kv_dT = lat.tile([128, NLC, s], mdt, tag="kvd")   # [L, s]
        q_dT = lat.tile([128, NLC, s], mdt, tag="qd")     # [L, s]
        k_rT = lat.tile([128, 2, s], xdt, tag="krt")      # rotated rope-k

        # ================= PHASE A: down/rope projections ================
        with ExitStack() as pa:
            hstp = pa.enter_context(tc.tile_pool(name="hst", bufs=2))
            wdp = pa.enter_context(tc.tile_pool(name="wdown", bufs=1))
            psa = pa.enter_context(tc.tile_pool(name="psa", bufs=1,
                                                space="PSUM"))
            rkr = pa.enter_context(tc.tile_pool(name="rkr", bufs=2))

            w_down_t = wdp.tile([128, NKC, 512], adt, tag="wd")
            w_rk_t = wdp.tile([128, NKC, HPC * RD], adt, tag="wr")
            wd_r = w_down.rearrange("(ko p) m -> p ko m", p=128)
            wr_r = w_rk.rearrange("(ko p) m -> p ko m", p=128)

            names = ("kv0", "kv1", "q0", "q1")
            for sb in range(nsb):
                sbs = slice(sb * SB, (sb + 1) * SB)
                hst_t = hstp.tile([128, NKC, SB], adt, tag="hsT")
                if sb == 0:
                    # fine-grained first block so the first matmuls start
                    # as early as possible; weights interleaved per-kc on
                    # the gpsimd queue
                    for kc in range(NKC):
                        nc.sync.dma_start(hst_t[:, kc, :],
                                          hst_r[:, kc, sbs])
                        nc.gpsimd.dma_start(w_down_t[:, kc, :],
                                            wd_r[:, kc, :])
                        nc.gpsimd.dma_start(w_rk_t[:, kc, :],
                                            wr_r[:, kc, :])
                    nc.gpsimd.dma_start(cc_t[:], rope_cc[:])
                    nc.gpsimd.dma_start(ss_t[:], rope_ss[:])
                    nc.gpsimd.dma_start(
                        w_qc_t[:], w_qc.rearrange("(ko p) m -> p ko m", p=128))
                    nc.gpsimd.dma_start(
                        w_qr_t[:], w_qr.rearrange("(ko p) m -> p ko m", p=128))
                    nc.gpsimd.dma_start(
                        w_ku_t[:], w_ku.rearrange("(ko p) m -> p ko m", p=128))
                    nc.gpsimd.dma_start(
                        w_vu_t[:], w_vu.rearrange("(ko p) m -> p ko m", p=128))
                else:
                    for j in range(4):
                        nc.sync.dma_start(hst_t[:, 4 * j:4 * j + 4, :],
                                          hst_r[:, 4 * j:4 * j + 4, sbs])

                pb = {n: psa.tile([128, SB], f32, tag=f"psa_{n}",
                                  name=f"psa_{n}_{sb}") for n in names}
                # kr banks double-buffered: the rope rotate reads them after
                # the cos/sin tables land, which can lag a block behind
                pkr0 = psa.tile([128, SB], f32, tag="psa_kr0", bufs=2,
                                name=f"psa_kr0_{sb}")
                pkr1 = psa.tile([128, SB], f32, tag="psa_kr1", bufs=2,
                                name=f"psa_kr1_{sb}")

                for kc in range(NKC):
                    rhs = hst_t[:, kc, :]
                    st = kc == 0
                    sp = kc == NKC - 1
                    nc.tensor.matmul(pb["kv0"][:], w_down_t[:, kc, 0:128],
                                     rhs, start=st, stop=sp)
                    nc.tensor.matmul(pb["kv1"][:], w_down_t[:, kc, 128:256],
                                     rhs, start=st, stop=sp)
                    nc.tensor.matmul(pb["q0"][:], w_down_t[:, kc, 256:384],
                                     rhs, start=st, stop=sp)
                    nc.tensor.matmul(pb["q1"][:], w_down_t[:, kc, 384:512],
                                     rhs, start=st, stop=sp)
                    nc.tensor.matmul(pkr0[:], w_rk_t[:, kc, 0:128],
                                     rhs, start=st, stop=sp)
                    nc.tensor.matmul(pkr1[:], w_rk_t[:, kc, 128:256],
                                     rhs, start=st, stop=sp)

                nc.scalar.copy(kv_dT[:, 0, sbs], pb["kv0"][:])
                nc.vector.tensor_copy(kv_dT[:, 1, sbs], pb["kv1"][:])
                nc.scalar.copy(q_dT[:, 0, sbs], pb["q0"][:])
                nc.vector.tensor_copy(q_dT[:, 1, sbs], pb["q1"][:])

                # rotate rope-k straight out of psum:
                #   lo' = lo*cos - hi*sin ; hi' = hi*cos + lo*sin
                r1 = rkr.tile([128, SB], f32, tag="r1")
                r4 = rkr.tile([128, SB], f32, tag="r4")
                nc.vector.tensor_mul(r1[:], pkr0[:], cc_t[:, sbs])
                nc.vector.tensor_mul(r4[:], pkr0[:], ss_t[:, sbs])
                r2 = rkr.tile([128, SB], f32, tag="r2")
                r3 = rkr.tile([128, SB], f32, tag="r3")
                nc.vector.tensor_mul(r2[:], pkr1[:], ss_t[:, sbs])
                nc.vector.tensor_mul(r3[:], pkr1[:], cc_t[:, sbs])
                nc.vector.tensor_sub(k_rT[:, 0, sbs], r1[:], r2[:])
                nc.vector.tensor_add(k_rT[:, 1, sbs], r3[:], r4[:])

        # ================= B/C1/C2/D share one psum pool =================
        qkp_cm = tc.tile_pool(name="qkp", bufs=1, side="right")
        qkp = qkp_cm.__enter__()
        vp_cm = tc.tile_pool(name="vp", bufs=1, side="right")
        vp = vp_cm.__enter__()
        yp_cm = tc.tile_pool(name="yp", bufs=1, side="right")
        yp = yp_cm.__enter__()
        qT = {h: qkp.tile([128, s], xdt, tag=f"qT{h}", name=f"qT{h}")
              for h in range(HPC)}
        kT = {h: qkp.tile([128, s], xdt, tag=f"kT{h}", name=f"kT{h}")
              for h in range(HPC)}
        # flat v: columns = [sc chunk][2 heads x 128]
        v_pair = {hp: vp.tile([128, nsc * 2 * HD], xdt, tag=f"vp{hp}",
                              name=f"vp{hp}")
                  for hp in range(2)}
        yT_all = yp.tile([128, HPC, s], xdt, tag="yT")

        psm_cm = tc.tile_pool(name="psm", bufs=1, space="PSUM")
        psm = psm_cm.__enter__()
        rqp_cm = tc.tile_pool(name="rqp", bufs=2)
        rqp = rqp_cm.__enter__()
        qrr_cm = tc.tile_pool(name="qrr", bufs=2)
        qrr = qrr_cm.__enter__()

        def emit_b(hp):
            """v for a pair: [128,1024] blk psums, 4 chunks each."""
            for g in range(nsc // 4):
                pv = psm.tile([128, 1024], f32, tag="blk", bufs=2,
                              name=f"pv_{hp}_{g}")
                # lc inner: a 256-col region must finish accumulating
                # before its bank-sibling starts (start=True clears the
                # has_written bits of the whole bank)
                for j in range(4):
                    sc = 4 * g + j
                    for lc in range(NLC):
                        nc.tensor.matmul(
                            pv[:, j * 256:(j + 1) * 256],
                            kv_dT[:, lc, sc * 128:(sc + 1) * 128],
                            w_vu_t[:, lc, hp * 256:(hp + 1) * 256],
                            start=(lc == 0), stop=(lc == NLC - 1))
                vs = slice(g * 1024, (g + 1) * 1024)
                # pair 0 runs at the tail of phase A (DVE busy with the
                # rope-k rotate) -> ACT; pair 1 runs inside the attention
                # stream (ACT busy with exps) -> DVE
                if hp == 0:
                    nc.scalar.copy(v_pair[hp][:, vs], pv[:])
                else:
                    nc.vector.tensor_copy(v_pair[hp][:, vs], pv[:])

        def emit_c1_rope(hp, sb2):
            """q-rope chain for one 1024-block: psum -> DVE rotate ->
            DMA scatter.  psum rows = [h0_lo, h1_lo, h0_hi, h1_hi]."""
            h0, h1 = 2 * hp, 2 * hp + 1
            ws = slice(sb2 * 1024, (sb2 + 1) * 1024)
            pr = psm.tile([128, 1024], f32, tag="blk", bufs=2,
                          name=f"pr_{hp}_{sb2}")
            for half in range(2):
                hs_ = slice(sb2 * 1024 + half * 512,
                            sb2 * 1024 + (half + 1) * 512)
                for lc in range(NLC):
                    nc.tensor.matmul(
                        pr[:, half * 512:(half + 1) * 512],
                        w_qr_t[:, lc, hp * 128:(hp + 1) * 128],
                        q_dT[:, lc, hs_],
                        start=(lc == 0), stop=(lc == NLC - 1))
            # t13 = pr*cos (SBUF), t24 = pr*sin (PSUM so the cross-base
            # addsub below is legal), then pair-packed rotate + scatter
            t13 = rqp.tile([128, 1024], f32, tag="t13")
            nc.vector.tensor_mul(t13[:], pr[:], cc_t[:, ws])
            t24 = psm.tile([128, 1024], f32, tag="blk", bufs=2,
                           name=f"t24_{hp}_{sb2}")
            nc.vector.tensor_mul(t24[:], pr[:], ss_t[:, ws])
            qr_rot = qrr.tile([128, 1024], xdt, tag="qr")
            nc.vector.tensor_sub(qr_rot[0:64, :], t13[0:64, :],
                                 t24[64:128, :])
            nc.vector.tensor_add(qr_rot[64:128, :], t13[64:128, :],
                                 t24[0:64, :])
            for h in (h0, h1):
                hl = 32 * (h - h0)
                nc.sync.dma_start(qT[h][64:96, ws], qr_rot[hl:hl + 32, :])
                nc.sync.dma_start(qT[h][96:128, ws],
                                  qr_rot[64 + hl:64 + hl + 32, :])

        def emit_c1_k(hp, sb2):
            """k content + rope-k scatter for one 1024-block."""
            h0, h1 = 2 * hp, 2 * hp + 1
            ws = slice(sb2 * 1024, (sb2 + 1) * 1024)
            pk = psm.tile([128, 1024], f32, tag="blk", bufs=2,
                          name=f"pk_{hp}_{sb2}")
            for half in range(2):
                hs_ = slice(sb2 * 1024 + half * 512,
                            sb2 * 1024 + (half + 1) * 512)
                for lc in range(NLC):
                    nc.tensor.matmul(
                        pk[:, half * 512:(half + 1) * 512],
                        w_ku_t[:, lc, hp * 128:(hp + 1) * 128],
                        kv_dT[:, lc, hs_],
                        start=(lc == 0), stop=(lc == NLC - 1))
            nc.scalar.copy(kT[h0][0:64, ws], pk[0:64, :])
            nc.vector.tensor_copy(kT[h1][0:64, ws], pk[64:128, :])
            for h in (h0, h1):
                rb = slice(32 * h, 32 * h + 32)
                nc.sync.dma_start(kT[h][64:96, ws], k_rT[rb, 0, ws])
                nc.sync.dma_start(kT[h][96:128, ws], k_rT[rb, 1, ws])

        def emit_c1_q(hp, sb2):
            """q content for one 1024-block."""
            h0, h1 = 2 * hp, 2 * hp + 1
            ws = slice(sb2 * 1024, (sb2 + 1) * 1024)
            pc = psm.tile([128, 1024], f32, tag="blk", bufs=2,
                          name=f"pc_{hp}_{sb2}")
            for half in range(2):
                hs_ = slice(sb2 * 1024 + half * 512,
                            sb2 * 1024 + (half + 1) * 512)
                for lc in range(NLC):
                    nc.tensor.matmul(
                        pc[:, half * 512:(half + 1) * 512],
                        w_qc_t[:, lc, hp * 128:(hp + 1) * 128],
                        q_dT[:, lc, hs_],
                        start=(lc == 0), stop=(lc == NLC - 1))
            nc.scalar.copy(qT[h0][0:64, ws], pc[0:64, :])
            nc.vector.tensor_copy(qT[h1][0:64, ws], pc[64:128, :])

        wo_tiles = []

        def emit_tail_prep():
            """Close the pair-prep pools and prefetch the o-proj weights
            (emitted once, after pair 1's C1 pieces)."""
            qrr_cm.__exit__(None, None, None)
            rqp_cm.__exit__(None, None, None)
            lat_cm.__exit__(None, None, None)
            wsmall_cm.__exit__(None, None, None)
            rope_cm.__exit__(None, None, None)
            wop = wop_cm.__enter__()
            for ncol in range(H // 512):
                wo_t = wop.tile([128, HPC, 512], xdt, tag=f"wo{ncol}")
                nc.gpsimd.dma_start(
                    wo_t[:],
                    w_o[:, ncol * 512:(ncol + 1) * 512].rearrange(
                        "(ho p) m -> p ho m", p=128))
                wo_tiles.append(wo_t)

        wop_cm = tc.tile_pool(name="wop", bufs=1, side="right")

        # pair 0 prep up front
        emit_b(0)
        for sb2 in range(nsb2):
            emit_c1_rope(0, sb2)
        for sb2 in range(nsb2):
            emit_c1_k(0, sb2)
            emit_c1_q(0, sb2)

        # pair 1 prep, emitted piecewise inside pair 0's unit stream
        pieces = []
        for sb2 in range(nsb2):
            pieces.append(lambda sb2=sb2: emit_c1_rope(1, sb2))
        for sb2 in range(nsb2):
            pieces.append(lambda sb2=sb2: emit_c1_k(1, sb2))
            pieces.append(lambda sb2=sb2: emit_c1_q(1, sb2))
        pieces.append(lambda: emit_b(1))
        pieces.append(emit_tail_prep)

        # ================= PHASE C2: causal attention ====================
        nqb = s // SB
        dpq = SB // KB
        esp_cm = tc.tile_pool(name="esp", bufs=5, side="right")
        esp = esp_cm.__enter__()
        recp_cm = tc.tile_pool(name="recp", bufs=2, side="right")
        recp = recp_cm.__enter__()

        outp_cm = tc.tile_pool(name="outp", bufs=4, side="right")
        outp = outp_cm.__enter__()

        def emit_d_chunk(sc):
            poa = psm.tile([128, 1024], f32, tag="blk", bufs=2,
                           name=f"poa_{sc}")
            pob = psm.tile([128, 512], f32, tag="py", bufs=2,
                           name=f"pob_{sc}")
            poc = psm.tile([128, 512], f32, tag="pden", bufs=2,
                           name=f"poc_{sc}")
            regions = [poa[:, 0:512], poa[:, 512:1024], pob[:], poc[:]]
            for hh in range(HPC):
                for ncol in range(H // 512):
                    nc.tensor.matmul(
                        regions[ncol],
                        yT_all[:, hh, sc * 128:(sc + 1) * 128],
                        wo_tiles[ncol][:, hh, :],
                        start=(hh == 0), stop=(hh == HPC - 1))
            ot = outp.tile([128, 2048], bf16, tag="ot", name=f"ot_{sc}")
            nc.scalar.copy(ot[:, 0:1024], poa[:])
            nc.vector.tensor_copy(ot[:, 1024:1536], pob[:])
            nc.vector.tensor_copy(ot[:, 1536:2048], poc[:])
            nc.sync.dma_start(
                out[sc * 128:(sc + 1) * 128, 0:1024], ot[:, 0:1024])
            nc.sync.dma_start(
                out[sc * 128:(sc + 1) * 128, 1024:2048], ot[:, 1024:2048])

        units = []
        for hp in range(2):
            for qi in range(nqb):
                for kjp in range((qi + 1) * dpq // 2):
                    for h in (2 * hp, 2 * hp + 1):
                        units.append((hp, qi, h, kjp))
        n_p0 = sum(1 for u in units if u[0] == 0)
        py = {}
        pden = {}

        def score2(u):
            hp, qi, h, kjp = u
            ps2 = psm.tile([128, 1024], f32, tag="blk", bufs=2,
                           name=f"ps_{h}_{qi}_{kjp}")
            qs = slice(qi * SB, (qi + 1) * SB)
            for j in range(2):
                kj = 2 * kjp + j
                nc.tensor.matmul(
                    ps2[:, j * 512:(j + 1) * 512],
                    kT[h][:, kj * KB:(kj + 1) * KB],
                    qT[h][:, qs], start=True, stop=True)
            return ps2

        def expmask(u, ps2):
            hp, qi, h, kjp = u
            es = esp.tile([128, 1024], xdt, tag="es",
                          name=f"es_{h}_{qi}_{kjp}")
            nc.scalar.activation(es[:], ps2[:], Exp, scale=SCALE)
            for j in range(2):
                kj = 2 * kjp + j
                if kj >= qi * dpq:          # diagonal block
                    nc.gpsimd.affine_select(
                        out=es[:, j * 512:(j + 1) * 512],
                        in_=es[:, j * 512:(j + 1) * 512],
                        compare_op=is_ge, fill=0.0,
                        base=qi * SB - kj * KB,
                        pattern=[[1, SB]],
                        channel_multiplier=-1)
            return es

        def avden(u, es):
            hp, qi, h, kjp = u
            nkj = (qi + 1) * dpq
            if kjp == 0:        # lazy: hide the ring wait
                py[(qi, h)] = psm.tile(
                    [128, SB], f32, tag="py", bufs=2, name=f"py_{h}_{qi}")
                pden[(qi, h)] = psm.tile(
                    [128, SB], f32, tag="pden", bufs=2,
                    name=f"pden_{h}_{qi}")
            for j in range(2):
                kj = 2 * kjp + j
                esj = es[:, j * 512:(j + 1) * 512]
                vcol = kj * 256 + (h - 2 * hp) * HD
                nc.tensor.matmul(
                    py[(qi, h)][:],
                    v_pair[hp][:, vcol:vcol + HD],
                    esj, start=(kj == 0), stop=(kj == nkj - 1))
                nc.tensor.matmul(
                    pden[(qi, h)][:], ones128[:], esj,
                    start=(kj == 0), stop=(kj == nkj - 1))
            if kjp == nkj // 2 - 1:     # last unit of (qi, h)
                qs = slice(qi * SB, (qi + 1) * SB)
                rec = recp.tile([128, SB], f32, tag="rec",
                                name=f"rec_{h}_{qi}")
                nc.vector.reciprocal_approx_fast(rec[:], pden[(qi, h)][:])
                nc.vector.tensor_mul(yT_all[:, h, qs],
                                     py[(qi, h)][:], rec[:])
                if hp == 1 and h == 2 * hp + 1:
                    # pair 1's (qi) rows done for all 4 heads -> its four
                    # o-proj chunks can stream out during the remaining
                    # attention units
                    d_ready.extend(range(4 * qi, 4 * qi + 4))

        pend_exp = []   # scored, waiting exp (depth 1)
        pend_av = []    # exp'd, waiting AV/den (depth 4)
        d_ready = []    # o-proj chunks whose yT rows are complete
        # interleave pair-1 prep pieces among pair-0's units, finishing
        # well before the first pair-1 unit needs their outputs
        piece_at = {6 + 3 * i: p for i, p in enumerate(pieces)}
        assert not piece_at or max(piece_at) < n_p0 - 4
        for ui, u in enumerate(units):
            if ui in piece_at:
                piece_at[ui]()
            elif d_ready:
                emit_d_chunk(d_ready.pop(0))
            pend_exp.append((u, score2(u)))
            if len(pend_exp) > 1:
                uu, pp = pend_exp.pop(0)
                pend_av.append((uu, expmask(uu, pp)))
            if len(pend_av) > 3:
                avden(*pend_av.pop(0))
        for it in pend_exp:
            pend_av.append((it[0], expmask(it[0], it[1])))
        for it in pend_av:
            avden(*it)
        for sc in d_ready:
            emit_d_chunk(sc)

        if DEBUG:
            f32c = lambda ap: ap.bitcast(f32) if xdt == f32r else ap
            nc.sync.dma_start(dbg["d_yT"][:], f32c(yT_all[:]))

        # (phase D chunks were interleaved into the unit stream above)

        psm_cm.__exit__(None, None, None)
        wop_cm.__exit__(None, None, None)
        outp_cm.__exit__(None, None, None)
        recp_cm.__exit__(None, None, None)
        esp_cm.__exit__(None, None, None)
        yp_cm.__exit__(None, None, None)
        vp_cm.__exit__(None, None, None)
        qkp_cm.__exit__(None, None, None)

    nc.compile()
    return nc


# ======================= host-side preparation ==========================

def _rope_tables(s):
    inv_freq = 1.0 / (ROPE_BASE ** (np.arange(0, RD, 2, dtype=np.float64) / RD))
    t = np.arange(s, dtype=np.float64)
    freqs = np.outer(t, inv_freq)                    # [s, 32]
    cc = np.tile(np.cos(freqs).T, (4, 1)).astype(np.float32)   # [128, s]
    ss = np.tile(np.sin(freqs).T, (4, 1)).astype(np.float32)
    return np.ascontiguousarray(cc), np.ascontiguousarray(ss)


def make_in_maps(hidden_states, Wkv_d, Wq_d, Wk_u, Wq_u, Wv_u, Wrk, Wrq, Wo,
                 s=S, a_bf16=A_BF16, attn_bf16=ATTN_BF16):
    f32 = np.float32
    import ml_dtypes
    a_np = ml_dtypes.bfloat16 if a_bf16 else f32
    x_np = ml_dtypes.bfloat16 if attn_bf16 else f32
    w_down = np.ascontiguousarray(
        np.concatenate([Wkv_d, Wq_d], axis=1), dtype=a_np)      # [H, 512]
    rope_cc, rope_ss = _rope_tables(s)
    Wk_u4 = Wk_u.reshape(L, NH, RD)
    Wq_u4 = Wq_u.reshape(L, NH, RD)
    Wrq4 = Wrq.reshape(L, NH, RD)
    Wv_u4 = Wv_u.reshape(L, NH, HD)
    Wrk4 = Wrk.reshape(H, NH, RD)
    Wo4 = Wo.reshape(NH, HD, H)

    def pack_lo_hi(w4, hsel, dim0):
        # [dim0, 4 heads, 64] -> cols [h0_lo..h3_lo, h0_hi..h3_hi]
        wl = w4[:, hsel, 0:RD // 2]                  # [d, 4, 32]
        wh = w4[:, hsel, RD // 2:RD]
        return np.ascontiguousarray(np.concatenate(
            [wl.reshape(dim0, HPC * 32), wh.reshape(dim0, HPC * 32)],
            axis=1), dtype=a_np)                     # [d, 256]

    def pack_qr_pairs(w4, hsel):
        # per pair p: [h(2p)_lo, h(2p+1)_lo, h(2p)_hi, h(2p+1)_hi] (32 each)
        cols = []
        heads = list(range(hsel.start, hsel.stop))
        for p in range(2):
            ha, hb = heads[2 * p], heads[2 * p + 1]
            cols.extend([w4[:, ha, 0:32], w4[:, hb, 0:32],
                         w4[:, ha, 32:64], w4[:, hb, 32:64]])
        return np.ascontiguousarray(
            np.concatenate(cols, axis=1), dtype=f32)  # [L, 256]

    in_maps = []
    hsT = {b: np.ascontiguousarray(hidden_states[b, :s].T, dtype=a_np)
           for b in range(B)}
    for c in range(8):
        b, g = divmod(c, 4)
        hsel = slice(g * HPC, (g + 1) * HPC)
        in_maps.append({
            "hst": hsT[b],
            "w_down": w_down,
            "w_rk": pack_lo_hi(Wrk4, hsel, H),
            "w_qc": np.ascontiguousarray(
                Wq_u4[:, hsel, :].reshape(L, HPC * RD), dtype=f32),
            "w_qr": pack_qr_pairs(Wrq4, hsel),
            "w_ku": np.ascontiguousarray(
                Wk_u4[:, hsel, :].reshape(L, HPC * RD), dtype=f32),
            "w_vu": np.ascontiguousarray(
                Wv_u4[:, hsel, :].reshape(L, HPC * HD), dtype=f32),
            "w_o": np.ascontiguousarray(
                Wo4[hsel].reshape(HPC * HD, H), dtype=x_np),
            "rope_cc": rope_cc,
            "rope_ss": rope_ss,
        })
    return in_maps


_NC_CACHE = {}


def kernel(hidden_states, Wkv_d, Wq_d, Wk_u, Wq_u, Wv_u, Wrk, Wrq, Wo):
    from concourse.bass_utils import run_bass_kernel_spmd

    key = (S, MM_DTYPE, A_BF16, ATTN_BF16)
    if key not in _NC_CACHE:
        _NC_CACHE[key] = build_nc(S, MM_DTYPE, A_BF16, ATTN_BF16)
    nc = _NC_CACHE[key]

    in_maps = make_in_maps(
        np.asarray(hidden_states), np.asarray(Wkv_d), np.asarray(Wq_d),
        np.asarray(Wk_u), np.asarray(Wq_u), np.asarray(Wv_u),
        np.asarray(Wrk), np.asarray(Wrq), np.asarray(Wo))

    res = run_bass_kernel_spmd(nc, in_maps, core_ids=list(range(8)))
    parts = [np.asarray(r["out"], dtype=np.float32) for r in res.results]
    out = np.empty((B, S, H), dtype=np.float32)
    for b in range(B):
        out[b] = parts[4 * b] + parts[4 * b + 1] + parts[4 * b + 2] + parts[4 * b + 3]
    return out
